# revision 2
# baseline (speedup 1.0000x reference)
"""Trainium2 Bass kernel for nn_Attention_59030030516520 (v2).

Fused attention block: qkv projection + per-head RMSNorm + segmented RoPE +
softmax attention + output projection, distributed over 8 NeuronCores as
batch(2) x head-groups(4).  Each core computes 4 heads of one batch element
and a partial output projection; the host sums the partials and adds the bias.

v2 redesign vs v1:
- All matmuls run in bf16 (within the 2e-2 rel-err budget): full PE rate at
  any moving-dim size, 2x DVE element-wise throughput, half the DMA bytes.
- attn@V is computed with P^T chunks stationary (out[q,d] orientation): the
  moving dim drops 512->65 (PE time halves) and softmax row-sums land in a
  phantom ones-column of V'.  Normalization becomes a native per-partition
  tensor_scalar (q is the partition dim) -- no partition broadcasts.
- Exp is evaluated on [128,1024] PSUM chunks (2 k-tiles at a time) to
  amortize the ACT access latency; ACT exp is the ~135us/core floor that
  everything else (scores, attn@V, proj, next-tile qkv) hides behind.
- Attention output is transposed back to feature-major with cheap PE
  transposes against a bf16 identity, then projected; partial projection
  outputs stream out as fp16.
"""
import sys
sys.path.insert(0, "/opt/trn_rl_repo")
import numpy as np
import ml_dtypes
import concourse.bass as bass
import concourse.mybir as mybir
import concourse.tile as tile
from concourse import bacc

F32 = mybir.dt.float32
BF16 = mybir.dt.bfloat16
F16 = mybir.dt.float16
AF = mybir.ActivationFunctionType
ALU = mybir.AluOpType

B, N, C = 2, 2048, 1024
H, D = 16, 64
HPC = 4            # heads per core
NT = N // 128      # 16 seq tiles
QC = N // 512      # 4 q-chunks
EPS = 1e-6
SCALE = 1.0 / np.sqrt(D)
ROPE_SEGMENTS = (1024, 512)
NROPE = 1536
ROPE_THETA = 10000.0

BF = ml_dtypes.bfloat16


def build_kernel(w_is_ones=True, M=1):
    nc = bacc.Bacc("TRN2", target_bir_lowering=False, debug=False)

    # ---- DRAM I/O (per-core) ----
    xT_d = nc.dram_tensor("xT", [128, 8 * N], BF16, kind="ExternalInput")     # x[b].T packed [p, ci, n]
    wqkT_d = nc.dram_tensor("wqkT", [128, 8 * 512], BF16, kind="ExternalInput")  # [p, ci, f]
    bqk_d = nc.dram_tensor("bqk", [128, 4], F32, kind="ExternalInput")         # q,k bias per feature tile
    wvT_d = nc.dram_tensor("wvT", [128, 8 * 260], BF16, kind="ExternalInput")  # [p, ci, f] + phantom cols
    bv_d = nc.dram_tensor("bv", [128, 260], F32, kind="ExternalInput")         # v bias row bcast + ones at phantom
    cosF_d = nc.dram_tensor("cosF", [128, N], BF16, kind="ExternalInput")
    sinF_d = nc.dram_tensor("sinF", [128, N], BF16, kind="ExternalInput")
    wq_d = nc.dram_tensor("wq", [128, 1], F32, kind="ExternalInput")           # qn_w tiled
    wk_d = nc.dram_tensor("wk", [128, 1], F32, kind="ExternalInput")
    ind_d = nc.dram_tensor("ind", [128, 33], BF16, kind="ExternalInput")       # 64-row group indicator
    ident_d = nc.dram_tensor("ident", [128, 128], BF16, kind="ExternalInput")  # transpose identity
    wpT_d = nc.dram_tensor("wpT", [256, C], BF16, kind="ExternalInput")        # proj weights slice.T
    yT_d = nc.dram_tensor("yT", [C, N], F16, kind="ExternalOutput")            # partial proj out.T (fp16)

    with tile.TileContext(nc) as tc:
        with (
            nc.allow_low_precision(reason="bf16 attention within 2e-2 rel-err budget"),
            tc.tile_pool(name="pers", bufs=1) as pers,      # persistent tensors (unique tags)
            tc.tile_pool(name="work", bufs=4) as work,      # rope/stats working tiles
            tc.tile_pool(name="vp", bufs=16) as vpool,      # v' tiles live through attention
            tc.tile_pool(name="p2", bufs=25) as p2pool,     # exp(scores^T) bf16 tiles
            tc.tile_pool(name="abf", bufs=8) as abfpool,    # normalized attn out [q, d-major]
            tc.tile_pool(name="atp", bufs=3) as atpool,     # transposed attn out (sbuf)
            tc.tile_pool(name="sm", bufs=4) as sm,          # small tiles (rcp)
            tc.tile_pool(name="yo", bufs=3) as yopool,      # proj psum->sbuf staging
            tc.tile_pool(name="psA", bufs=2, space="PSUM") as psA,  # [128,1024] qkv/stats/scores
            tc.tile_pool(name="psO", bufs=4, space="PSUM") as psO,  # [128,512] v/po/transpose/proj
        ):
          for _rep in range(M):
            # ---- load weights/constants ----
            # One big DMA per packed tensor: DMA-issue instructions cost
            # >1.2us of sequencer time each, so few big beats many small.
            # The ACT queue issues NO dmas (it must start biasing by ~18us).
            wqkTr = pers.tile([128, 8 * 512], BF16, tag="wqkTr", name="wqkTr")
            nc.sync.dma_start(wqkTr[:], wqkT_d[:])
            bqk = pers.tile([128, 4], F32, tag="bqk")
            nc.gpsimd.dma_start(bqk[:], bqk_d[:])
            xTr = pers.tile([128, 8 * N], BF16, tag="xTr", name="xTr")
            nc.sync.dma_start(xTr[:], xT_d[:])
            cosF = pers.tile([128, N], BF16, tag="cosF")
            nc.gpsimd.dma_start(cosF[:], cosF_d[:])
            sinF = pers.tile([128, N], BF16, tag="sinF")
            nc.gpsimd.dma_start(sinF[:], sinF_d[:])
            wvTr = pers.tile([128, 8 * 260], BF16, tag="wvTr", name="wvTr")
            nc.gpsimd.dma_start(wvTr[:], wvT_d[:])
            wpT = [pers.tile([128, C], BF16, tag=f"wp{i}", name=f"wp{i}") for i in range(2)]
            for i in range(2):
                nc.sync.dma_start(wpT[i][:], wpT_d[128 * i:128 * (i + 1), :])
            bv = pers.tile([128, 260], F32, tag="bv")
            nc.gpsimd.dma_start(bv[:], bv_d[:])
            wq = pers.tile([128, 1], F32, tag="wq")
            nc.sync.dma_start(wq[:], wq_d[:])
            wk = pers.tile([128, 1], F32, tag="wk")
            nc.sync.dma_start(wk[:], wk_d[:])
            ind = pers.tile([128, 33], BF16, tag="ind")
            nc.gpsimd.dma_start(ind[:], ind_d[:])
            ident = pers.tile([128, 128], BF16, tag="ident")
            nc.sync.dma_start(ident[:], ident_d[:])
            eps_t = pers.tile([64, 1], F32, tag="eps", name="eps_t")
            nc.vector.memset(eps_t[:], EPS)

            # qkf[0],qkf[1] = roped+normed q heads (0,1),(2,3); [2],[3] = k
            qkf = [pers.tile([128, N], BF16, tag=f"qkf{t}", name=f"qkf{t}") for t in range(4)]
            raw = [pers.tile([128, N], BF16, tag=f"raw{t}", name=f"raw{t}") for t in range(4)]

            ones64 = pers.tile([64, 64], BF16, tag="ones64", name="ones64")
            nc.vector.memset(ones64[:], 1.0)
            scr1 = pers.tile([64, 1], F32, tag="scr1", name="scr1")
            # preload the Sqrt ACT table while DMAs stream in
            nc.scalar.activation(scr1[0:1, 0:1], eps_t[0:1, 0:1], AF.Sqrt)
            # PE warmup: junk matmuls on the (already-landed) weight tile keep
            # the PE pstate ramp running while xTr streams in, so qkv runs at
            # full clock from its first matmul.
            for w in range(42):
                wps = psO.tile([128, 512], F32, tag="pO", name=f"warm{w}")
                nc.tensor.matmul(wps[:], wqkTr[:, 0:128], wqkTr[:, 0:512],
                                 start=True, stop=True)

            # ---- qkv: one 128-feature tile [feature, seq] ----
            # bias+copy on ACT for early tiles (ACT idle), DVE for late tiles
            # (ACT busy with the exp stream by then)
            def qkv_tile(t, bias_dve=False):
                for half in range(2):
                    ps = psA.tile([128, 1024], F32, tag="sA", name=f"qk{t}_{half}")
                    for ci in range(8):
                        for q2 in range(2):
                            qc4 = 2 * half + q2
                            nc.tensor.matmul(
                                ps[:, 512 * q2:512 * (q2 + 1)],
                                wqkTr[:, 512 * ci + 128 * t:512 * ci + 128 * (t + 1)],
                                xTr[:, N * ci + 512 * qc4:N * ci + 512 * (qc4 + 1)],
                                start=(ci == 0), stop=(ci == 7),
                            )
                    if bias_dve:
                        nc.vector.tensor_scalar(
                            raw[t][:, 1024 * half:1024 * (half + 1)], ps[:],
                            bqk[:, t:t + 1], None, ALU.add)
                    else:
                        nc.scalar.activation(
                            raw[t][:, 1024 * half:1024 * (half + 1)], ps[:],
                            AF.Identity, bias=bqk[:, t:t + 1])

            # ---- RMSNorm stats for one tile: sq -> stats-mm -> sqrt -> recip ----
            irbs = [None] * 4
            def stats(t):
                sq = work.tile([128, N], BF16, tag="work", name=f"sq{t}")
                nc.vector.tensor_tensor(sq[:], raw[t][:], raw[t][:], ALU.mult)
                ir = work.tile([64, N], F32, tag="ir", bufs=1, name=f"ir{t}")
                for half in range(2):
                    pr = psA.tile([128, 1024], F32, tag="sA", name=f"pr{t}_{half}")
                    for q2 in range(2):
                        nc.tensor.matmul(
                            pr[0:33, 512 * q2:512 * (q2 + 1)], ind[:],
                            sq[:, 1024 * half + 512 * q2:1024 * half + 512 * (q2 + 1)],
                            start=True, stop=True)
                    nc.scalar.activation(
                        ir[0:33, 1024 * half:1024 * (half + 1)],
                        pr[0:33, :1024], AF.Sqrt,
                        bias=eps_t[0:33], scale=1.0 / D)
                irb = work.tile([64, N], BF16, tag="irb", bufs=4, name=f"irb{t}")
                nc.vector.reciprocal(irb[0:33, :], ir[0:33, :])
                irbs[t] = irb

            # ---- RoPE + norm scaling for one tile ----
            # ir rows are broadcast to 64-row blocks with tiny K=1 PE matmuls
            # into PSUM; the qkf scaling multiply reads the PSUM broadcast.
            def rope(t):
                irb = irbs[t]
                if not w_is_ones:
                    wvec = wq if t < 2 else wk
                    nc.vector.tensor_scalar(raw[t][:], raw[t][:], wvec[:], None, ALU.mult)
                # swap halves of each head's d-dims for the rotate-half term
                sw = work.tile([128, NROPE], BF16, tag="work", name=f"sw{t}")
                for blk in range(4):
                    sfrom = (blk // 2) * 64 + (32 if blk % 2 == 0 else 0)
                    sto = (blk // 2) * 64 + (0 if blk % 2 == 0 else 32)
                    [nc.gpsimd, nc.sync][blk % 2].dma_start(
                        sw[sto:sto + 32, :], raw[t][sfrom:sfrom + 32, 0:NROPE])
                ropeo = work.tile([128, NROPE], BF16, tag="work", name=f"ro{t}")
                nc.vector.tensor_tensor(ropeo[:], raw[t][:, 0:NROPE], cosF[:, 0:NROPE], ALU.mult)
                nc.vector.tensor_tensor(sw[:], sw[:], sinF[:, 0:NROPE], ALU.mult)
                nc.vector.tensor_tensor(ropeo[:], ropeo[:], sw[:], ALU.add)
                for c in range(4):
                    bcp = psO.tile([128, 512], F32, tag="pO", name=f"bc{t}_{c}")
                    nc.tensor.matmul(bcp[0:64, :], ones64[0:1, :],
                                     irb[0:1, 512 * c:512 * (c + 1)],
                                     start=True, stop=True)
                    nc.tensor.matmul(bcp[64:128, :], ones64[32:33, :],
                                     irb[32:33, 512 * c:512 * (c + 1)],
                                     start=True, stop=True)
                    src = (ropeo[:, 512 * c:512 * (c + 1)] if c < 3
                           else raw[t][:, NROPE:N])
                    nc.vector.tensor_tensor(
                        qkf[t][:, 512 * c:512 * (c + 1)], bcp[:], src, ALU.mult)

            # ---- v' tiles: [seq, 65*4] with ones columns, bf16 ----
            vp = [None] * NT
            def v_tile(st):
                ps = psO.tile([128, 512], F32, tag="pO", name=f"v{st}")
                for ci in range(8):
                    nc.tensor.matmul(
                        ps[:, :260],
                        xTr[:, N * ci + 128 * st:N * ci + 128 * (st + 1)],
                        wvTr[:, 260 * ci:260 * (ci + 1)],
                        start=(ci == 0), stop=(ci == 7),
                    )
                v = vpool.tile([128, 260], BF16, tag="v", name=f"vb{st}")
                nc.vector.tensor_tensor(v[:], ps[:, :260], bv[:], ALU.add)
                vp[st] = v

            # ---- attention: scores+exp for one (head, qc) ----
            # p2 tiles: [128 k, 1024] = exp(scores^T) for 2 k-tiles (bf16)
            def attn_S(qc, hl, p2s):
                ti, ro = hl // 2, 64 * (hl % 2)
                qf, kf = qkf[ti], qkf[2 + ti]
                for kp in range(8):
                    s2 = psA.tile([128, 1024], F32, tag="sA", name=f"s{qc}_{hl}_{kp}")
                    for k2 in range(2):
                        kc = 2 * kp + k2
                        nc.tensor.matmul(
                            s2[:, 512 * k2:512 * (k2 + 1)],
                            kf[ro:ro + 64, 128 * kc:128 * (kc + 1)],
                            qf[ro:ro + 64, 512 * qc:512 * (qc + 1)],
                            start=True, stop=True,
                        )
                    p2 = p2pool.tile([128, 1024], BF16, tag="p2", name=f"p{qc}_{hl}_{kp}")
                    nc.scalar.activation(p2[:], s2[:], AF.Exp, scale=float(SCALE))
                    p2s.append(p2)

            # ---- attention: attn@V + normalize for one (head, qc) ----
            def attn_A(qc, hl, p2s, abf):
                # one accumulation chain at a time per PSUM bank: qt outer
                po = psO.tile([128, 512], F32, tag="pO", name=f"po{qc}_{hl}")
                for qt in range(4):
                    for kp in range(8):
                        p2 = p2s[kp]
                        for k2 in range(2):
                            kc = 2 * kp + k2
                            nc.tensor.matmul(
                                po[:, 65 * qt:65 * qt + 65],
                                p2[:, 512 * k2 + 128 * qt:512 * k2 + 128 * (qt + 1)],
                                vp[kc][:, 65 * hl:65 * (hl + 1)],
                                start=(kc == 0), stop=(kc == 15),
                            )
                rcp = sm.tile([128, 4], F32, tag="rcp", name=f"rc{qc}_{hl}")
                nc.vector.reciprocal(rcp[:], po[:, 64:260:65])
                for qt in range(4):
                    nc.vector.tensor_scalar(
                        abf[qt][:, 64 * hl:64 * (hl + 1)],
                        po[:, 65 * qt:65 * qt + 64],
                        rcp[:, qt:qt + 1], None, ALU.mult)

            def attn_SA(qc, hl, abf):
                p2s = []
                attn_S(qc, hl, p2s)
                attn_A(qc, hl, p2s, abf)

            # ---- qc epilogue: transpose to feature-major + projection ----
            def finish_qc(qc, abf):
                aTs = []
                for pair in range(2):
                    tp = psO.tile([128, 512], BF16, tag="pO", name=f"tp{qc}_{pair}")
                    for qt in range(4):
                        for hh in range(2):
                            hl = 2 * pair + hh
                            nc.tensor.transpose(
                                tp[64 * hh:64 * (hh + 1), 128 * qt:128 * (qt + 1)],
                                abf[qt][:, 64 * hl:64 * (hl + 1)],
                                ident[:])
                    aT = atpool.tile([128, 512], BF16, tag="aT", name=f"aT{qc}_{pair}")
                    nc.vector.tensor_copy(aT[:], tp[:])
                    aTs.append(aT)
                for ot in range(8):
                    yp = psO.tile([128, 512], F32, tag="pO", name=f"yp{qc}_{ot}")
                    for c2 in range(2):
                        nc.tensor.matmul(
                            yp[:, :512],
                            wpT[c2][:, 128 * ot:128 * (ot + 1)],
                            aTs[c2][:],
                            start=(c2 == 0), stop=(c2 == 1),
                        )
                    yo = yopool.tile([128, 512], F16, tag="yo", name=f"yo{qc}_{ot}")
                    nc.vector.tensor_copy(yo[:], yp[:, :512])
                    nc.sync.dma_start(
                        yT_d[128 * ot:128 * (ot + 1), 512 * qc:512 * (qc + 1)],
                        yo[:])

            # ---- emission order (engines execute in-order; this IS the
            # schedule).  Pipeline shape:
            #   lead-in: tiles 0,2 (q heads 0/1 + their k) -> first exps ~27us
            #   S00,S01 keep ACT busy while PE does tiles 3,1 + v'
            #   steady state: two S blocks in flight ahead of their A blocks
            qkv_tile(0); qkv_tile(2)
            stats(0); stats(2)
            rope(0); rope(2)
            qkv_tile(3); qkv_tile(1)
            stats(3); stats(1)
            rope(3); rope(1)
            abf0 = [abfpool.tile([128, 256], BF16, tag="abf", name=f"ab0_{qt}")
                    for qt in range(4)]
            p2s_pre = [[], [], [], []]
            attn_S(0, 0, p2s_pre[0])
            attn_S(0, 1, p2s_pre[1])
            attn_S(0, 2, p2s_pre[2])
            for st in range(NT):
                v_tile(st)
            attn_A(0, 0, p2s_pre[0], abf0)
            attn_A(0, 1, p2s_pre[1], abf0)
            attn_S(0, 3, p2s_pre[3])
            attn_A(0, 2, p2s_pre[2], abf0)
            attn_A(0, 3, p2s_pre[3], abf0)
            finish_qc(0, abf0)
            for qc in range(1, QC):
                abf = [abfpool.tile([128, 256], BF16, tag="abf", name=f"ab{qc}_{qt}")
                       for qt in range(4)]
                for pair in range(2):
                    ph = [[], []]
                    attn_S(qc, 2 * pair, ph[0])
                    attn_S(qc, 2 * pair + 1, ph[1])
                    attn_A(qc, 2 * pair, ph[0], abf)
                    attn_A(qc, 2 * pair + 1, ph[1], abf)
                finish_qc(qc, abf)

    nc.compile()
    return nc


# ---------------- host-side data prep ----------------

def rope_tables():
    inv_freq = 1.0 / (ROPE_THETA ** (np.arange(0, D, 2, dtype=np.float32) / D))  # [32]
    cos = np.ones((32, N), np.float32)
    sin = np.zeros((32, N), np.float32)
    start = 0
    for seg in ROPE_SEGMENTS:
        ang = np.arange(seg, dtype=np.float32)[None, :] * inv_freq[:, None]  # [32, seg]
        cos[:, start:start + seg] = np.cos(ang)
        sin[:, start:start + seg] = np.sin(ang)
        start += seg
    cosF = np.empty((128, N), np.float32)
    sinF = np.empty((128, N), np.float32)
    for hp in range(2):
        r = 64 * hp
        cosF[r:r + 32] = cos; cosF[r + 32:r + 64] = cos
        sinF[r:r + 32] = -sin; sinF[r + 32:r + 64] = sin
    return cosF.astype(BF), sinF.astype(BF)


def core_inputs(core, x, qkv_w, qkv_b, qn_w, kn_w, proj_w):
    b, g = divmod(core, 4)
    heads = [4 * g + i for i in range(HPC)]
    xT = np.ascontiguousarray(x[b].T).astype(BF).reshape(8, 128, N).transpose(1, 0, 2).reshape(128, 8 * N)
    q_rows = np.concatenate([np.arange(64 * h, 64 * h + 64) for h in heads])
    k_rows = q_rows + C
    v_rows = q_rows + 2 * C
    qk_rows = np.concatenate([q_rows, k_rows])
    wqkT = (np.ascontiguousarray(qkv_w[qk_rows].T).astype(BF)
            .reshape(8, 128, 512).transpose(1, 0, 2).reshape(128, 8 * 512))
    bqk = np.ascontiguousarray(qkv_b[qk_rows].reshape(4, 128).T)    # [128, 4]
    wvT = np.zeros((C, 260), np.float32)
    bv = np.zeros((260,), np.float32)
    for hl in range(HPC):
        wvT[:, 65 * hl:65 * hl + 64] = qkv_w[v_rows[64 * hl:64 * hl + 64]].T
        bv[65 * hl:65 * hl + 64] = qkv_b[v_rows[64 * hl:64 * hl + 64]]
        bv[65 * hl + 64] = 1.0
    bv128 = np.broadcast_to(bv, (128, 260)).copy()
    cosF, sinF = rope_tables()
    wq = np.tile(qn_w.astype(np.float32), 2)[:, None].copy()  # [128,1]
    wk = np.tile(kn_w.astype(np.float32), 2)[:, None].copy()
    ind = np.zeros((128, 33), np.float32)
    ind[0:64, 0] = 1.0; ind[64:128, 32] = 1.0
    ident = np.eye(128, dtype=np.float32).astype(BF)
    wpT = np.ascontiguousarray(proj_w[:, 256 * g:256 * (g + 1)].T).astype(BF)  # [256, C]
    wvTp = (wvT.astype(BF).reshape(8, 128, 260).transpose(1, 0, 2)
            .reshape(128, 8 * 260))
    return {
        "xT": xT, "wqkT": wqkT, "bqk": bqk, "wvT": wvTp, "bv": bv128,
        "cosF": cosF, "sinF": sinF, "wq": wq, "wk": wk,
        "ind": ind.astype(BF), "ident": ident, "wpT": wpT,
    }


def gather(results, proj_b):
    y = np.empty((B, N, C), np.float32)
    for b in range(B):
        acc = np.zeros((C, N), np.float32)
        for g in range(4):
            acc += results[4 * b + g]["yT"].astype(np.float32)
        y[b] = acc.T + proj_b[None, :]
    return y


class Runner:
    """Compiled SPMD runner (jit once, execute many) mirroring run_bass_via_pjrt."""

    def __init__(self, nc, n_cores=8):
        import jax
        import numpy as _np
        from jax.sharding import Mesh, PartitionSpec
        from jax.experimental.shard_map import shard_map
        import concourse.mybir as _mybir
        from concourse.bass2jax import _bass_exec_p, install_neuronx_cc_hook, partition_id_tensor

        install_neuronx_cc_hook()
        self.n_cores = n_cores
        partition_name = nc.partition_id_tensor.name if nc.partition_id_tensor else None
        in_names, out_names, out_avals, zero_outs = [], [], [], []
        for alloc in nc.m.functions[0].allocations:
            if not isinstance(alloc, _mybir.MemoryLocationSet):
                continue
            name = alloc.memorylocations[0].name
            if alloc.kind == "ExternalInput":
                if name != partition_name:
                    in_names.append(name)
            elif alloc.kind == "ExternalOutput":
                out_names.append(name)
                shape = tuple(alloc.tensor_shape)
                dtype = _mybir.dt.np(alloc.dtype)
                out_avals.append(jax.core.ShapedArray(shape, dtype))
                zero_outs.append(_np.zeros(shape, dtype))
        self.in_names, self.out_names = in_names, out_names
        self.out_avals, self.zero_outs = out_avals, zero_outs
        n_params, n_outs = len(in_names), len(out_avals)
        self.n_params = n_params
        all_in_names = list(in_names) + list(out_names)
        if partition_name is not None:
            all_in_names.append(partition_name)

        def _body(*args):
            operands = list(args)
            if partition_name is not None:
                operands.append(partition_id_tensor())
            outs = _bass_exec_p.bind(
                *operands,
                out_avals=tuple(out_avals),
                in_names=tuple(all_in_names),
                out_names=tuple(out_names),
                lowering_input_output_aliases=(),
                sim_require_finite=True,
                sim_require_nnan=True,
                nc=nc,
            )
            return tuple(outs)

        devices = jax.devices()[:n_cores]
        mesh = Mesh(_np.asarray(devices), ("core",))
        in_specs = (PartitionSpec("core"),) * (n_params + n_outs)
        out_specs = (PartitionSpec("core"),) * n_outs
        self._fn = jax.jit(
            shard_map(_body, mesh=mesh, in_specs=in_specs, out_specs=out_specs,
                      check_rep=False),
            keep_unused=True,
        )
        self._jax = jax

    def prep(self, in_maps):
        import numpy as _np
        per_core = [[_np.asarray(m[nm]) for nm in self.in_names] for m in in_maps]
        concat_in = [
            _np.concatenate([per_core[c][i] for c in range(self.n_cores)], axis=0)
            for i in range(self.n_params)
        ]
        concat_zeros = [
            _np.zeros((self.n_cores * z.shape[0], *z.shape[1:]), z.dtype)
            for z in self.zero_outs
        ]
        return concat_in + concat_zeros

    def run_device(self, dev_args):
        outs = self._fn(*dev_args)
        self._jax.block_until_ready(outs)
        return outs

    def run(self, in_maps):
        import numpy as _np
        outs = self.run_device(self.prep(in_maps))
        return [
            {nm: _np.asarray(outs[i]).reshape(self.n_cores, *self.out_avals[i].shape)[c]
             for i, nm in enumerate(self.out_names)}
            for c in range(self.n_cores)
        ]


_CACHE = {}


def _get_kernel(w_is_ones, M=1):
    key = (bool(w_is_ones), M)
    if key not in _CACHE:
        nc = build_kernel(w_is_ones=key[0], M=M)
        _CACHE[key] = (nc, Runner(nc, 8))
    return _CACHE[key]


def kernel(x, qkv_w, qkv_b, qn_w, kn_w, proj_w, proj_b):
    x = np.ascontiguousarray(np.asarray(x, dtype=np.float32))
    qkv_w = np.ascontiguousarray(np.asarray(qkv_w, dtype=np.float32))
    qkv_b = np.ascontiguousarray(np.asarray(qkv_b, dtype=np.float32))
    qn_w = np.ascontiguousarray(np.asarray(qn_w, dtype=np.float32))
    kn_w = np.ascontiguousarray(np.asarray(kn_w, dtype=np.float32))
    proj_w = np.ascontiguousarray(np.asarray(proj_w, dtype=np.float32))
    proj_b = np.ascontiguousarray(np.asarray(proj_b, dtype=np.float32))
    w_is_ones = bool(np.all(qn_w == 1.0) and np.all(kn_w == 1.0))
    nc, runner = _get_kernel(w_is_ones)
    in_maps = [core_inputs(c, x, qkv_w, qkv_b, qn_w, kn_w, proj_w)
               for c in range(8)]
    results = runner.run(in_maps)
    return gather(results, proj_b)
